# revision 1
# baseline (speedup 1.0000x reference)
"""Trainium2 Bass kernel for modded-nanogpt CausalSelfAttention, 8-way
tensor-parallel over heads.

Per core (4 of 32 heads, local width F=128):
  - QKV projection in [t, f] layout: lhsT = x^T d-tiles (stationary),
    rhs = [wq|wk|wv]^T slice -> psum [t128, 384].
  - RMS-norm + half-truncated rotary on q,k via DVE; rsqrt = exp(-0.5*ln(x))
    so the whole kernel uses one ACT table set (natural_log_exp).
  - q,k transposed to [f, t] with DMA-transpose; v merged with lambdas*ve and
    stored augmented with a ones column ([v_h | 1] per head) for softmax sums.
  - Scores S^T[j, i] per head with K=32 row-tiled matmuls (4 heads concurrent
    in the PE array via tile_position), exp on ACT (scale=0.12 folded in),
    causal handled block-granular + triangle mask on diagonal blocks.
  - attn@v: lhsT = vaug (M=33: 32 v cols + ones), rhs = P^T -> y^T and the
    softmax denominator in one accumulation; normalize with DVE reciprocal +
    DMA partition-broadcast.
  - c_proj on the local 128 e-columns -> partial [t, d] output; partials are
    summed across the 8 cores on the host.
"""
import os
import sys

if os.path.isdir("/opt/trn_rl_repo") and "/opt/trn_rl_repo" not in sys.path:
    sys.path.insert(0, "/opt/trn_rl_repo")

from contextlib import ExitStack

import ml_dtypes
import numpy as np

import concourse.bass as bass
import concourse.tile as tile
from concourse import bacc, mybir
from concourse.bass_utils import run_bass_kernel_spmd

F32 = mybir.dt.float32
BF16 = mybir.dt.bfloat16
AF = mybir.ActivationFunctionType
ALU = mybir.AluOpType

N_CORES = 8
T = 2048
D = 1024
HD = 32          # head dim
HC = 4           # heads per core
F = HC * HD      # local qkv width = 128
TT = T // 128    # 16 t-tiles
DT = D // 128    # 8 d-tiles
CW = 512         # i-chunk width
NCH = T // CW    # 4 i-chunks
SCALE = 0.12
RMS_EPS = 1.1920929e-07
ROT_BASE = HD * 4


def _ap(a, offset_delta, dims):
    """Re-strided view of tile AP `a`: keep partition dim, replace free dims."""
    return bass.AP(tensor=a.tensor, offset=a.offset + offset_delta,
                   ap=[list(a.ap[0])] + [list(d) for d in dims])


def _bcast_part(a, n):
    """View of AP `a` with the partition dim replaced by an n-way broadcast."""
    return bass.AP(tensor=a.tensor, offset=a.offset,
                   ap=[[0, n]] + [list(d) for d in a.ap[1:]])


def build_program(repeats: int = 1):
    nc = bacc.Bacc("TRN2", target_bir_lowering=False, debug=False,
                   num_devices=N_CORES)

    xt = nc.dram_tensor("xt", [D, T], BF16, kind="ExternalInput").ap()
    wqkv = nc.dram_tensor("wqkv", [D, 3 * F], BF16, kind="ExternalInput").ap()
    vein = nc.dram_tensor("vein", [T, F], F32, kind="ExternalInput").ap()
    cpw = nc.dram_tensor("cpw", [F, D], BF16, kind="ExternalInput").ap()
    lam = nc.dram_tensor("lam", [128, 2], F32, kind="ExternalInput").ap()
    rota = nc.dram_tensor("rota", [T, HD], BF16, kind="ExternalInput").ap()
    rotb = nc.dram_tensor("rotb", [T, HD], BF16, kind="ExternalInput").ap()
    trimask = nc.dram_tensor("trimask", [128, 128], BF16,
                             kind="ExternalInput").ap()
    out = nc.dram_tensor("out", [T, D], F32, kind="ExternalOutput").ap()

    with tile.TileContext(nc) as tc, ExitStack() as ctx:
        const = ctx.enter_context(tc.tile_pool(name="const", bufs=1))
        x_sb = const.tile([128, DT, T], BF16)
        w_sb = const.tile([128, DT, 3 * F], BF16)
        cpw_sb = const.tile([128, D], BF16)
        lam_sb = const.tile([128, 2], F32)
        rota_sb = const.tile([128, TT, HD], BF16)
        rotb_sb = const.tile([128, TT, HD], BF16)
        tri_sb = const.tile([128, 128], BF16)
        eps_sb = const.tile([128, 1], F32)
        qT_sb = const.tile([128, T], BF16)
        kT_sb = const.tile([128, T], BF16)
        vaug_sb = const.tile([128, TT, 33 * HC], BF16)
        yT_sb = const.tile([128, T], BF16)

        def body(_iv=None):
            # ---- constant loads ----
            nc.sync.dma_start(x_sb[:], xt.rearrange("(n p) t -> p n t", p=128))
            nc.sync.dma_start(w_sb[:], wqkv.rearrange("(n p) f -> p n f", p=128))
            nc.sync.dma_start(cpw_sb[:], cpw)
            nc.sync.dma_start(lam_sb[:], lam)
            nc.sync.dma_start(rota_sb[:], rota.rearrange("(n p) f -> p n f", p=128))
            nc.sync.dma_start(rotb_sb[:], rotb.rearrange("(n p) f -> p n f", p=128))
            nc.sync.dma_start(tri_sb[:], trimask)
            nc.vector.memset(eps_sb[:], RMS_EPS)
            for h in range(HC):
                nc.vector.memset(vaug_sb[:, :, 33 * h + 32:33 * h + 33], 1.0)

            qk_all = const.tile([128, TT, 2 * F], BF16)
            ms_all = const.tile([128, TT, 8], F32)
            rs_all = const.tile([128, TT, 8], F32)
            with tc.tile_pool(name="qkv_ps", bufs=2, space="PSUM") as qkv_ps, \
                 tc.tile_pool(name="work", bufs=3) as work:
                # pass 1: projections + squared-sum stats + v merge
                for tt in range(TT):
                    ps = qkv_ps.tile([128, 3 * F], F32)
                    for dt in range(DT):
                        nc.tensor.matmul(
                            ps[:], lhsT=x_sb[:, dt, 128 * tt:128 * (tt + 1)],
                            rhs=w_sb[:, dt, :],
                            start=(dt == 0), stop=(dt == DT - 1))
                    nc.vector.tensor_copy(qk_all[:, tt, :], ps[:, 0:2 * F])
                    sq = work.tile([128, 2 * F], F32, tag="sq")
                    nc.vector.tensor_mul(sq[:], qk_all[:, tt, :],
                                         qk_all[:, tt, :])
                    nc.vector.reduce_sum(
                        ms_all[:, tt, :],
                        sq[:].rearrange("p (g d) -> p g d", g=8),
                        axis=mybir.AxisListType.X)

                    # --- v: lam0*v + lam1*ve, augmented layout ---
                    vet = work.tile([128, F], F32, tag="vet")
                    nc.sync.dma_start(vet[:], vein[128 * tt:128 * (tt + 1), :])
                    vel = work.tile([128, F], F32, tag="vel")
                    nc.vector.tensor_scalar_mul(vel[:], vet[:], lam_sb[:, 1:2])
                    vaug_v = _ap(vaug_sb[:, tt, :], 0, [[33, HC], [1, HD]])
                    nc.vector.scalar_tensor_tensor(
                        out=vaug_v, in0=ps[:, 2 * F:3 * F],
                        scalar=lam_sb[:, 0:1], in1=vel[:],
                        op0=ALU.mult, op1=ALU.add)

                # pass 2: one batched ln + exp -> rstd for all tiles.
                # Single Ln before every Exp in the program => one ACT
                # table load (natural_log_exp set covers both).
                nc.scalar.activation(rs_all[:], ms_all[:], AF.Ln,
                                     bias=eps_sb[:], scale=1.0 / HD)
                nc.scalar.activation(rs_all[:], rs_all[:], AF.Exp, scale=-0.5)

                # pass 3: rotary + norm-apply + transpose to [f, t]
                for tt in range(TT):
                    qk = qk_all[:, tt, :]
                    ta = work.tile([128, 2 * F], BF16, tag="ta")
                    rota_v = _ap(rota_sb[:, tt, :], 0, [[0, 8], [1, HD]])
                    nc.vector.tensor_mul(ta[:], qk, rota_v)
                    tb = work.tile([128, 2 * F], BF16, tag="tb")
                    qk_swap = _ap(qk, 16, [[32, 8], [-16, 2], [1, 16]])
                    rotb_v = _ap(rotb_sb[:, tt, :], 0, [[0, 8], [1, HD]])
                    nc.vector.tensor_mul(tb[:], qk_swap, rotb_v)
                    rot = work.tile([128, 2 * F], BF16, tag="rot")
                    nc.vector.tensor_add(rot[:], ta[:], tb[:])
                    qkn = work.tile([128, 2 * F], BF16, tag="qkn")
                    rs_v = _ap(rs_all[:, tt, :], 0, [[1, 8], [0, HD]])
                    nc.vector.tensor_mul(qkn[:], rot[:], rs_v)

                    nc.sync.dma_start_transpose(
                        qT_sb[:, 128 * tt:128 * (tt + 1)], qkn[:, 0:F])
                    nc.sync.dma_start_transpose(
                        kT_sb[:, 128 * tt:128 * (tt + 1)], qkn[:, F:2 * F])

            # ---- attention ----
            with tc.tile_pool(name="sc_ps", bufs=4, space="PSUM") as sc_ps, \
                 tc.tile_pool(name="y_ps", bufs=1, space="PSUM") as y_ps_pool, \
                 tc.tile_pool(name="pt", bufs=4) as pt_pool, \
                 tc.tile_pool(name="nrm", bufs=4) as nrm:
                for c in range(NCH):
                    y_ps = [y_ps_pool.tile([33, CW], F32, name=f"y{h}",
                                           tag=f"y{h}")
                            for h in range(HC)]
                    njt = 4 * c + 4
                    for jt in range(njt):
                        q0 = max(0, jt - 4 * c)
                        nt = CW - 128 * q0
                        i_lo = CW * c + 128 * q0
                        for h in range(HC):
                            sc = sc_ps.tile([128, CW], F32, tag="sc")
                            pt = pt_pool.tile([128, CW], BF16, tag="pt")
                            nc.tensor.matmul(
                                sc[:, 0:nt],
                                lhsT=kT_sb[32 * h:32 * (h + 1),
                                           128 * jt:128 * (jt + 1)],
                                rhs=qT_sb[32 * h:32 * (h + 1), i_lo:CW * (c + 1)],
                                start=True, stop=True,
                                tile_position=(32 * h, 0))
                            nc.scalar.activation(pt[:, 0:nt], sc[:, 0:nt],
                                                 AF.Exp, scale=SCALE)
                            if jt >= 4 * c:
                                nc.gpsimd.tensor_mul(pt[:, 0:128],
                                                     pt[:, 0:128], tri_sb[:])
                            nc.tensor.matmul(
                                y_ps[h][:, 128 * q0:CW],
                                lhsT=vaug_sb[:, jt, 33 * h:33 * (h + 1)],
                                rhs=pt[:, 0:nt],
                                start=(jt == 0), stop=(jt == njt - 1))
                    for h in range(HC):
                        rl = nrm.tile([1, CW], F32, tag="rl")
                        nc.vector.reciprocal(rl[:], y_ps[h][32:33, :])
                        rlb = nrm.tile([32, CW], F32, tag="rlb")
                        nc.gpsimd.partition_broadcast(rlb[:], rl[:])
                        nc.vector.tensor_mul(
                            yT_sb[32 * h:32 * (h + 1), CW * c:CW * (c + 1)],
                            y_ps[h][0:32, :], rlb[:])

            # ---- c_proj ----
            with tc.tile_pool(name="o_ps", bufs=4, space="PSUM") as o_ps, \
                 tc.tile_pool(name="o_sb", bufs=4) as o_sb:
                for ti in range(TT):
                    for dc in range(2):
                        op = o_ps.tile([128, CW], F32)
                        nc.tensor.matmul(
                            op[:], lhsT=yT_sb[:, 128 * ti:128 * (ti + 1)],
                            rhs=cpw_sb[:, CW * dc:CW * (dc + 1)],
                            start=True, stop=True)
                        ob = o_sb.tile([128, CW], F32, tag="ob")
                        nc.vector.tensor_copy(ob[:], op[:])
                        nc.sync.dma_start(
                            out[128 * ti:128 * (ti + 1), CW * dc:CW * (dc + 1)],
                            ob[:])

        if repeats == 1:
            body()
        else:
            with tc.For_i(0, repeats, 1) as _i:
                body(_i)

    nc.compile()
    return nc


def _host_inputs(x, ve, qkv_w, lambdas, c_proj_w):
    bf = ml_dtypes.bfloat16
    x2 = np.asarray(x, np.float32).reshape(T, D)
    xt = np.ascontiguousarray(x2.T).astype(bf)
    ve2 = np.asarray(ve, np.float32).reshape(T, D)
    qkv = np.asarray(qkv_w, np.float32)
    cp = np.asarray(c_proj_w, np.float32)
    lamv = np.asarray(lambdas, np.float32).reshape(2)
    lam_b = np.broadcast_to(lamv, (128, 2)).copy()

    quarter = HD // 4
    af = (np.float32(1.0) / np.float32(ROT_BASE)) ** np.linspace(
        0.0, 1.0, quarter, dtype=np.float32)
    af = np.concatenate([af, np.zeros(quarter, np.float32)])
    theta = np.arange(T, dtype=np.float32)[:, None] * af[None, :]
    cos = np.cos(theta).astype(np.float32)
    sin = np.sin(theta).astype(np.float32)
    rota = np.concatenate([cos, cos], axis=1).astype(bf)   # [T, 32]
    rotb = np.concatenate([sin, -sin], axis=1).astype(bf)  # [T, 32]

    jj = np.arange(128)[:, None]
    ii = np.arange(128)[None, :]
    trim = (ii >= jj).astype(bf)                       # keep j <= i

    in_maps = []
    for cidx in range(N_CORES):
        sl = slice(F * cidx, F * (cidx + 1))
        wq = qkv[0][sl, :]
        wk = qkv[1][sl, :]
        wv = qkv[2][sl, :]
        wcat = np.ascontiguousarray(
            np.concatenate([wq, wk, wv], axis=0).T).astype(bf)  # [1024, 384]
        in_maps.append({
            "xt": xt,
            "wqkv": wcat,
            "vein": np.ascontiguousarray(ve2[:, sl]),
            "cpw": np.ascontiguousarray(cp[:, sl].T).astype(bf),
            "lam": lam_b,
            "rota": rota,
            "rotb": rotb,
            "trimask": trim,
        })
    return in_maps


_NC_CACHE = {}


def _get_program(repeats: int = 1):
    if repeats not in _NC_CACHE:
        _NC_CACHE[repeats] = build_program(repeats)
    return _NC_CACHE[repeats]


def run_prepared(in_maps, repeats: int = 1):
    nc = _get_program(repeats)
    res = run_bass_kernel_spmd(nc, in_maps, list(range(N_CORES)))
    acc = np.zeros((T, D), np.float64)
    for cidx in range(N_CORES):
        acc += res.results[cidx]["out"].astype(np.float64)
    return acc.astype(np.float32).reshape(1, T, D)


def run(inputs, repeats: int = 1):
    nc = _get_program(repeats)
    in_maps = _host_inputs(**inputs)
    res = run_bass_kernel_spmd(nc, in_maps, list(range(N_CORES)))
    acc = np.zeros((T, D), np.float64)
    for cidx in range(N_CORES):
        acc += res.results[cidx]["out"].astype(np.float64)
    return acc.astype(np.float32).reshape(1, T, D)


def kernel(x, ve, qkv_w, lambdas, c_proj_w):
    return run(dict(x=x, ve=ve, qkv_w=qkv_w, lambdas=lambdas,
                    c_proj_w=c_proj_w))



# revision 17
# speedup vs baseline: 1.1726x; 1.1726x over previous
"""Trainium2 Bass kernel for modded-nanogpt CausalSelfAttention, 8-way
tensor-parallel over heads.

Per core (4 of 32 heads, local width F=128):
  - q,k projected DIRECTLY in [f, t] layout (lhsT = Wq^T/Wk^T d-tiles,
    rhs = x^T d-tiles) -> no transposes anywhere.  v projected in [t, f]
    (lhsT = x^T tile, rhs = Wv^T), lam0 folded into Wv and lam1 into ve
    on the host, merged with a single DVE add.
  - RMS stats via PE selector-matmul (partition reduction of q^2|k^2),
    rsqrt = exp(-0.5*ln(x)) on ACT (one table set for whole program),
    replicated to [128, t] via gpsimd partition_broadcast.
  - rotary via PE permutation matmul (swap-by-16 within head) + DVE
    muls with host cos/sin tables in [f, t] layout.
  - scores S^T[j, i] per head, K=32 row-tiled (tile_position=(32h,0), 4
    heads concurrent); exp on ACT batched over head PAIRS [128, 2*nt]
    (scale=0.12 folded in); causal: block-granular + triangle mask on
    diagonal blocks (gpsimd).
  - attn@v col-tiled: per head M=32 (tile_position=(0,32h)) into one
    [128, CW] psum tile; softmax denominators via col-tiled M=1
    ones-matmuls; normalize with DVE reciprocal + partition_broadcast.
  - c_proj per i-chunk (overlapped with later chunks) -> partial [t, d]
    summed across the 8 cores on the host.
"""
import os
import sys

if os.path.isdir("/opt/trn_rl_repo") and "/opt/trn_rl_repo" not in sys.path:
    sys.path.insert(0, "/opt/trn_rl_repo")

from contextlib import ExitStack

import ml_dtypes
import numpy as np

import concourse.bass as bass
import concourse.tile as tile
from concourse import bacc, mybir
from concourse.bass_utils import run_bass_kernel_spmd

F32 = mybir.dt.float32
BF16 = mybir.dt.bfloat16
AF = mybir.ActivationFunctionType
ALU = mybir.AluOpType

N_CORES = 8
T = 2048
D = 1024
HD = 32          # head dim
HC = 4           # heads per core
F = HC * HD      # local qkv width = 128
TT = T // 128    # 16 t-tiles
DT = D // 128    # 8 d-tiles
CW = 512         # i-chunk width
NCH = T // CW    # 4 i-chunks
SCALE = 0.12
RMS_EPS = 1.1920929e-07
ROT_BASE = HD * 4


def build_program(repeats: int = 1):
    nc = bacc.Bacc("TRN2", target_bir_lowering=False, debug=False,
                   num_devices=N_CORES)

    xt = nc.dram_tensor("xt", [D, T], BF16, kind="ExternalInput").ap()
    wq = nc.dram_tensor("wq", [D, F], BF16, kind="ExternalInput").ap()
    wk = nc.dram_tensor("wk", [D, F], BF16, kind="ExternalInput").ap()
    wv = nc.dram_tensor("wv", [D, F], BF16, kind="ExternalInput").ap()
    vein = nc.dram_tensor("vein", [T, F], BF16, kind="ExternalInput").ap()
    cpw = nc.dram_tensor("cpw", [F, D], BF16, kind="ExternalInput").ap()
    crep = nc.dram_tensor("crep", [F, T], BF16, kind="ExternalInput").ap()
    srep = nc.dram_tensor("srep", [F, T], BF16, kind="ExternalInput").ap()
    perm = nc.dram_tensor("perm", [128, 128], BF16, kind="ExternalInput").ap()
    selq = nc.dram_tensor("selq", [128, 128], BF16, kind="ExternalInput").ap()
    selk = nc.dram_tensor("selk", [128, 128], BF16, kind="ExternalInput").ap()
    selqb = nc.dram_tensor("selqb", [128, 128], BF16, kind="ExternalInput").ap()
    selkb = nc.dram_tensor("selkb", [128, 128], BF16, kind="ExternalInput").ap()
    trimask = nc.dram_tensor("trimask", [128, 128], BF16,
                             kind="ExternalInput").ap()
    out = nc.dram_tensor("out", [T, D], F32, kind="ExternalOutput").ap()
    dbg = {}
    if os.environ.get("KDBG"):
        for nm, shp in (("d_qT", [128, T]), ("d_kT", [128, T]),
                        ("d_v", [128, TT * F]), ("d_rs", [128, NCH * CW]),
                        ("d_yT", [128, T])):
            dbg[nm] = nc.dram_tensor(nm, shp, F32, kind="ExternalOutput").ap()

    with tile.TileContext(nc) as tc, ExitStack() as ctx:
        const = ctx.enter_context(tc.tile_pool(name="const", bufs=1))
        x_sb = const.tile([128, DT, T], BF16)
        wq_sb = const.tile([128, DT, F], BF16)
        wk_sb = const.tile([128, DT, F], BF16)
        wv_sb = const.tile([128, DT, F], BF16)
        ve_sb = const.tile([128, TT, F], BF16)
        cpw_sb = const.tile([128, D], BF16)
        crep_sb = const.tile([128, T], BF16)
        srep_sb = const.tile([128, T], BF16)
        perm_sb = const.tile([128, 128], BF16)
        selq_sb = const.tile([128, 128], BF16)
        selk_sb = const.tile([128, 128], BF16)
        selqb_sb = const.tile([128, 128], BF16)
        selkb_sb = const.tile([128, 128], BF16)
        tri_sb = const.tile([128, 128], BF16)
        ones_sb = const.tile([128, 1], BF16)
        eps_sb = const.tile([128, 1], F32)
        qT_sb = const.tile([128, T], BF16)
        kT_sb = const.tile([128, T], BF16)
        v_sb = const.tile([128, TT, F], BF16)
        yT_sb = const.tile([128, T], BF16)

        def body(_iv=None):
            # ---- constant loads ----
            xr = xt.rearrange("(n p) t -> p n t", p=128)
            for c in range(NCH):
                nc.sync.dma_start(x_sb[:, :, CW * c:CW * (c + 1)],
                                  xr[:, :, CW * c:CW * (c + 1)])
            nc.sync.dma_start(wq_sb[:], wq.rearrange("(n p) f -> p n f", p=128))
            nc.sync.dma_start(wk_sb[:], wk.rearrange("(n p) f -> p n f", p=128))
            nc.sync.dma_start(wv_sb[:], wv.rearrange("(n p) f -> p n f", p=128))
            nc.sync.dma_start(ve_sb[:], vein.rearrange("(n p) f -> p n f", p=128))
            nc.sync.dma_start(cpw_sb[:], cpw)
            nc.sync.dma_start(crep_sb[:], crep)
            nc.sync.dma_start(srep_sb[:], srep)
            nc.sync.dma_start(perm_sb[:], perm)
            nc.sync.dma_start(selq_sb[:], selq)
            nc.sync.dma_start(selk_sb[:], selk)
            nc.sync.dma_start(selqb_sb[:], selqb)
            nc.sync.dma_start(selkb_sb[:], selkb)
            nc.sync.dma_start(tri_sb[:], trimask)
            nc.vector.memset(ones_sb[:], 1.0)
            nc.vector.memset(eps_sb[:], RMS_EPS)

            # ---- QKV phase: per 512-col chunk ----
            rs_sb = const.tile([128, NCH, CW], BF16)
            with tc.tile_pool(name="qk_ps", bufs=1, space="PSUM") as qk_ps, \
                 tc.tile_pool(name="ms_ps", bufs=1, space="PSUM") as ms_ps_p, \
                 tc.tile_pool(name="sw_ps", bufs=1, space="PSUM") as sw_ps, \
                 tc.tile_pool(name="rep_ps", bufs=1, space="PSUM") as rep_ps, \
                 tc.tile_pool(name="qwork", bufs=2) as qwork:
                for c in range(NCH):
                    cs = slice(CW * c, CW * (c + 1))
                    q_ps = qk_ps.tile([128, CW], F32, tag="q")
                    k_ps = qk_ps.tile([128, CW], F32, tag="k")
                    v_ps = qk_ps.tile([128, HC, 128], F32, tag="v")
                    for dt in range(DT):
                        nc.tensor.matmul(q_ps[:], lhsT=wq_sb[:, dt, :],
                                         rhs=x_sb[:, dt, cs],
                                         start=(dt == 0), stop=(dt == DT - 1))
                    for dt in range(DT):
                        nc.tensor.matmul(k_ps[:], lhsT=wk_sb[:, dt, :],
                                         rhs=x_sb[:, dt, cs],
                                         start=(dt == 0), stop=(dt == DT - 1))
                    for st in range(HC):
                        t0 = CW * c + 128 * st
                        for dt in range(DT):
                            nc.tensor.matmul(
                                v_ps[:, st, :],
                                lhsT=x_sb[:, dt, t0:t0 + 128],
                                rhs=wv_sb[:, dt, :],
                                start=(dt == 0), stop=(dt == DT - 1))

                    # v merge: wv pre-scaled by lam0, ve by lam1 on host
                    nc.vector.tensor_add(
                        v_sb[:, HC * c:HC * (c + 1), :],
                        v_ps[:], ve_sb[:, HC * c:HC * (c + 1), :])

                    # raw q,k to SBUF (bf16)
                    qraw = qwork.tile([128, CW], BF16, tag="qraw")
                    kraw = qwork.tile([128, CW], BF16, tag="kraw")
                    nc.vector.tensor_copy(qraw[:], q_ps[:])
                    nc.vector.tensor_copy(kraw[:], k_ps[:])

                    # squared values packed [q | k]
                    sq = qwork.tile([128, 2 * CW], BF16, tag="sq")
                    nc.vector.tensor_mul(sq[:, 0:CW], qraw[:], qraw[:])
                    nc.vector.tensor_mul(sq[:, CW:2 * CW], kraw[:], kraw[:])

                    # partition-reduce within heads: ms[h, :] (q cols 0:CW,
                    # k cols CW:2CW)
                    ms_ps = ms_ps_p.tile([128, CW], F32, tag="ms")
                    nc.tensor.matmul(ms_ps[:], lhsT=selq_sb[:],
                                     rhs=sq[:, 0:CW], start=True, stop=False)
                    nc.tensor.matmul(ms_ps[:], lhsT=selk_sb[:],
                                     rhs=sq[:, CW:2 * CW],
                                     start=False, stop=True)

                    # rstd = exp(-0.5 * ln(ms/HD + eps))
                    lnr = qwork.tile([128, CW], F32, tag="lnr")
                    nc.scalar.activation(lnr[:], ms_ps[:], AF.Ln,
                                         bias=eps_sb[:], scale=1.0 / HD)
                    nc.scalar.activation(rs_sb[:, c, :], lnr[:], AF.Exp,
                                         scale=-0.5)

                    # replicate rstd to [128, CW] via PE selector matmuls
                    rep_q = rep_ps.tile([128, CW], F32, tag="rep_q")
                    rep_k = rep_ps.tile([128, CW], F32, tag="rep_k")
                    nc.tensor.matmul(rep_q[:], lhsT=selqb_sb[:],
                                     rhs=rs_sb[:, c, :], start=True, stop=True)
                    nc.tensor.matmul(rep_k[:], lhsT=selkb_sb[:],
                                     rhs=rs_sb[:, c, :], start=True, stop=True)

                    # rotary: swap-by-16 via PE permutation matmul
                    qsw_ps = sw_ps.tile([128, CW], F32, tag="qsw")
                    ksw_ps = sw_ps.tile([128, CW], F32, tag="ksw")
                    nc.tensor.matmul(qsw_ps[:], lhsT=perm_sb[:], rhs=qraw[:],
                                     start=True, stop=True)
                    nc.tensor.matmul(ksw_ps[:], lhsT=perm_sb[:], rhs=kraw[:],
                                     start=True, stop=True)

                    stage = os.environ.get("KSTAGE", "full")
                    for raw, swp, rep, dst in ((qraw, qsw_ps, rep_q, qT_sb),
                                               (kraw, ksw_ps, rep_k, kT_sb)):
                        if stage == "raw":
                            nc.vector.tensor_copy(dst[:, cs], raw[:])
                            continue
                        if stage == "norm":
                            nc.vector.tensor_mul(dst[:, cs], raw[:], rep[:])
                            continue
                        if stage == "swap":
                            nc.vector.tensor_copy(dst[:, cs], swp[:])
                            continue
                        ta = qwork.tile([128, CW], BF16, tag="ta")
                        tb = qwork.tile([128, CW], BF16, tag="tb")
                        rot = qwork.tile([128, CW], BF16, tag="rot")
                        nc.vector.tensor_mul(ta[:], raw[:], crep_sb[:, cs])
                        nc.vector.tensor_mul(tb[:], swp[:], srep_sb[:, cs])
                        nc.vector.tensor_add(rot[:], ta[:], tb[:])
                        if stage == "rot":
                            nc.vector.tensor_copy(dst[:, cs], rot[:])
                            continue
                        nc.vector.tensor_mul(dst[:, cs], rot[:], rep[:])

            # ---- attention + c_proj, per i-chunk ----
            with tc.tile_pool(name="sc_ps", bufs=2, space="PSUM") as sc_ps, \
                 tc.tile_pool(name="y_ps", bufs=1, space="PSUM") as y_ps_p, \
                 tc.tile_pool(name="r_ps", bufs=1, space="PSUM") as r_ps_p, \
                 tc.tile_pool(name="o_ps", bufs=1, space="PSUM") as o_ps_p, \
                 tc.tile_pool(name="rrb_ps", bufs=1, space="PSUM") as rrb_ps_p, \
                 tc.tile_pool(name="pt", bufs=4) as pt_pool, \
                 tc.tile_pool(name="nrm", bufs=2) as nrm, \
                 tc.tile_pool(name="o_sb", bufs=4) as o_sb:
                for c in range(NCH):
                    y_ps = y_ps_p.tile([128, CW], F32, tag="y")
                    r_ps = r_ps_p.tile([128, CW], F32, tag="r")
                    nc.vector.memset(r_ps[:], 1.0)
                    njt = 4 * c + 4
                    for jt in range(njt):
                        q0 = max(0, jt - 4 * c)
                        nt = CW - 128 * q0
                        i_lo = CW * c + 128 * q0
                        i_hi = CW * (c + 1)
                        diag = jt >= 4 * c
                        for p in range(2):
                            sc = sc_ps.tile([128, 2, CW], F32, tag="sc")
                            pt = pt_pool.tile([128, 2, CW], BF16, tag="pt")
                            for hh in range(2):
                                h = 2 * p + hh
                                nc.tensor.matmul(
                                    sc[:, hh, 0:nt],
                                    lhsT=kT_sb[32 * h:32 * (h + 1),
                                               128 * jt:128 * (jt + 1)],
                                    rhs=qT_sb[32 * h:32 * (h + 1), i_lo:i_hi],
                                    start=True, stop=True,
                                    tile_position=(32 * h, 0))
                            nc.scalar.activation(pt[:, :, 0:nt],
                                                 sc[:, :, 0:nt],
                                                 AF.Exp, scale=SCALE)
                            if diag:
                                for hh in range(2):
                                    nc.gpsimd.tensor_mul(
                                        pt[:, hh, 0:128],
                                        pt[:, hh, 0:128],
                                        tri_sb[:])
                            for hh in range(2):
                                h = 2 * p + hh
                                nc.tensor.matmul(
                                    y_ps[32 * h:32 * (h + 1), 128 * q0:CW],
                                    lhsT=v_sb[:, jt, 32 * h:32 * (h + 1)],
                                    rhs=pt[:, hh, 0:nt],
                                    start=(jt == 0), stop=(jt == njt - 1),
                                    tile_position=(0, 32 * h),
                                    skip_group_check=True)
                                nc.tensor.matmul(
                                    r_ps[32 * h:32 * h + 1, 128 * q0:CW],
                                    lhsT=ones_sb[:],
                                    rhs=pt[:, hh, 0:nt],
                                    start=(jt == 0), stop=(jt == njt - 1),
                                    tile_position=(0, 32 * h),
                                    skip_group_check=True)

                    # normalize: yT = y * (1/r) broadcast within heads
                    rr = nrm.tile([128, CW], BF16, tag="rr")
                    with nc.allow_low_precision(reason="softmax denom bf16"):
                        nc.vector.reciprocal(rr[:], r_ps[:])
                    rrb_ps = rrb_ps_p.tile([128, CW], F32, tag="rrb")
                    nc.tensor.matmul(rrb_ps[:], lhsT=selqb_sb[:], rhs=rr[:],
                                     start=True, stop=True)
                    rrb = nrm.tile([128, CW], BF16, tag="rrb")
                    nc.vector.tensor_copy(rrb[:], rrb_ps[:])
                    nc.vector.tensor_mul(yT_sb[:, CW * c:CW * (c + 1)],
                                         y_ps[:], rrb[:])

                    # c_proj for this i-chunk
                    for ti in range(4):
                        tg = 4 * c + ti
                        for dc in range(2):
                            op = o_ps_p.tile([128, CW], F32, tag="op")
                            nc.tensor.matmul(
                                op[:],
                                lhsT=yT_sb[:, 128 * tg:128 * (tg + 1)],
                                rhs=cpw_sb[:, CW * dc:CW * (dc + 1)],
                                start=True, stop=True)
                            ob = o_sb.tile([128, CW], F32, tag="ob")
                            nc.vector.tensor_copy(ob[:], op[:])
                            nc.sync.dma_start(
                                out[128 * tg:128 * (tg + 1),
                                    CW * dc:CW * (dc + 1)], ob[:])

            if dbg:
                with tc.tile_pool(name="dbgp", bufs=1) as dbgp:
                    for nm, sb in (("d_qT", qT_sb), ("d_kT", kT_sb),
                                   ("d_yT", yT_sb)):
                        dt_ = dbgp.tile([128, T], F32, tag=nm)
                        nc.vector.tensor_copy(dt_[:], sb[:])
                        nc.sync.dma_start(dbg[nm], dt_[:])
                    dt_ = dbgp.tile([128, TT, F], F32, tag="dv")
                    nc.vector.tensor_copy(dt_[:], v_sb[:])
                    nc.sync.dma_start(
                        dbg["d_v"].rearrange("p (a b) -> p a b", a=TT), dt_[:])
                    dt_ = dbgp.tile([128, NCH, CW], F32, tag="drs")
                    nc.vector.tensor_copy(dt_[:], rs_sb[:])
                    nc.sync.dma_start(
                        dbg["d_rs"].rearrange("p (a b) -> p a b", a=NCH),
                        dt_[:])

        if repeats == 1:
            body()
        else:
            with tc.For_i(0, repeats, 1) as _i:
                body(_i)

    nc.compile()
    return nc


def _host_inputs(x, ve, qkv_w, lambdas, c_proj_w):
    bf = ml_dtypes.bfloat16
    x2 = np.asarray(x, np.float32).reshape(T, D)
    xt = np.ascontiguousarray(x2.T).astype(bf)
    ve2 = np.asarray(ve, np.float32).reshape(T, D)
    qkv = np.asarray(qkv_w, np.float32)
    cp = np.asarray(c_proj_w, np.float32)
    lamv = np.asarray(lambdas, np.float32).reshape(2)

    # rotary tables in [f, t] layout, replicated per head
    quarter = HD // 4
    af = (np.float32(1.0) / np.float32(ROT_BASE)) ** np.linspace(
        0.0, 1.0, quarter, dtype=np.float32)
    af = np.concatenate([af, np.zeros(quarter, np.float32)])  # [16]
    theta = np.arange(T, dtype=np.float32)[:, None] * af[None, :]  # [T, 16]
    cos = np.cos(theta).astype(np.float32)   # [T, 16]
    sin = np.sin(theta).astype(np.float32)
    # crep[p, t] = cos(theta[t, p % 16]); srep sign-flipped on second half
    ph = np.arange(128) % HD                 # position within head
    crep = cos[:, ph % 16].T                 # [128, T]
    ssign = np.where(ph < 16, 1.0, -1.0).astype(np.float32)
    srep = (sin[:, ph % 16] * ssign[None, :]).T
    crep = np.ascontiguousarray(crep).astype(bf)
    srep = np.ascontiguousarray(srep).astype(bf)

    # swap-by-16 permutation within each 32-row head block:
    # out[p] = in[(p//32)*32 + ((p%32)+16)%32]  -> perm[j, p] = 1 iff j maps
    pm = np.zeros((128, 128), np.float32)
    for p in range(128):
        j = (p // 32) * 32 + ((p % 32) + 16) % 32
        pm[j, p] = 1.0
    perm = pm.astype(bf)

    # selectors: ms-reduce q->row 32h, k->row 32h+16; b = broadcast back
    sq_ = np.zeros((128, 128), np.float32)
    sk_ = np.zeros((128, 128), np.float32)
    for p in range(128):
        sq_[p, 32 * (p // 32)] = 1.0
        sk_[p, 32 * (p // 32) + 16] = 1.0
    selq_, selk_ = sq_.astype(bf), sk_.astype(bf)
    selqb_ = np.ascontiguousarray(sq_.T).astype(bf)
    selkb_ = np.ascontiguousarray(sk_.T).astype(bf)

    jj = np.arange(128)[:, None]
    ii = np.arange(128)[None, :]
    trim = (ii >= jj).astype(bf)                       # keep j <= i

    in_maps = []
    for cidx in range(N_CORES):
        sl = slice(F * cidx, F * (cidx + 1))
        wqh = np.ascontiguousarray(qkv[0][sl, :].T).astype(bf)  # [D, F]
        wkh = np.ascontiguousarray(qkv[1][sl, :].T).astype(bf)
        wvh = np.ascontiguousarray(
            (lamv[0] * qkv[2][sl, :]).T).astype(bf)
        in_maps.append({
            "xt": xt,
            "wq": wqh,
            "wk": wkh,
            "wv": wvh,
            "vein": np.ascontiguousarray(lamv[1] * ve2[:, sl]).astype(bf),
            "cpw": np.ascontiguousarray(cp[:, sl].T).astype(bf),
            "crep": crep,
            "srep": srep,
            "perm": perm,
            "selq": selq_,
            "selk": selk_,
            "selqb": selqb_,
            "selkb": selkb_,
            "trimask": trim,
        })
    return in_maps


_NC_CACHE = {}


def _get_program(repeats: int = 1):
    if repeats not in _NC_CACHE:
        _NC_CACHE[repeats] = build_program(repeats)
    return _NC_CACHE[repeats]


def run_prepared(in_maps, repeats: int = 1):
    nc = _get_program(repeats)
    res = run_bass_kernel_spmd(nc, in_maps, list(range(N_CORES)))
    acc = np.zeros((T, D), np.float64)
    for cidx in range(N_CORES):
        acc += res.results[cidx]["out"].astype(np.float64)
    return acc.astype(np.float32).reshape(1, T, D)


def run(inputs, repeats: int = 1):
    in_maps = _host_inputs(**inputs)
    return run_prepared(in_maps, repeats)


def kernel(x, ve, qkv_w, lambdas, c_proj_w):
    return run(dict(x=x, ve=ve, qkv_w=qkv_w, lambdas=lambdas,
                    c_proj_w=c_proj_w))


# revision 18
# speedup vs baseline: 1.4869x; 1.2681x over previous
"""Trainium2 Bass kernel for modded-nanogpt CausalSelfAttention, 8-way
tensor-parallel over heads.

Per core (4 of 32 heads, local width F=128):
  - q,k projected DIRECTLY in [f, t] layout (lhsT = Wq^T/Wk^T d-tiles,
    rhs = x^T d-tiles) -> no transposes anywhere.  v projected in [t, f]
    (lhsT = x^T tile, rhs = Wv^T), lam0 folded into Wv and lam1 into ve
    on the host, merged with a single DVE add.
  - RMS stats via PE selector-matmul (partition reduction of q^2|k^2),
    rsqrt = exp(-0.5*ln(x)) on ACT (one table set for whole program),
    replicated to [128, t] via gpsimd partition_broadcast.
  - rotary via PE permutation matmul (swap-by-16 within head) + DVE
    muls with host cos/sin tables in [f, t] layout.
  - scores S^T[j, i] per head, K=32 row-tiled (tile_position=(32h,0), 4
    heads concurrent); exp on ACT batched over head PAIRS [128, 2*nt]
    (scale=0.12 folded in); causal: block-granular + triangle mask on
    diagonal blocks (gpsimd).
  - attn@v col-tiled: per head M=32 (tile_position=(0,32h)) into one
    [128, CW] psum tile; softmax denominators via col-tiled M=1
    ones-matmuls; normalize with DVE reciprocal + partition_broadcast.
  - c_proj per i-chunk (overlapped with later chunks) -> partial [t, d]
    summed across the 8 cores on the host.
"""
import os
import sys

if os.path.isdir("/opt/trn_rl_repo") and "/opt/trn_rl_repo" not in sys.path:
    sys.path.insert(0, "/opt/trn_rl_repo")

from contextlib import ExitStack

import ml_dtypes
import numpy as np

import concourse.bass as bass
import concourse.tile as tile
from concourse import bacc, mybir
from concourse.bass_utils import run_bass_kernel_spmd

F32 = mybir.dt.float32
BF16 = mybir.dt.bfloat16
AF = mybir.ActivationFunctionType
ALU = mybir.AluOpType

N_CORES = 8
T = 2048
D = 1024
HD = 32          # head dim
HC = 4           # heads per core
F = HC * HD      # local qkv width = 128
TT = T // 128    # 16 t-tiles
DT = D // 128    # 8 d-tiles
CW = 512         # i-chunk width
NCH = T // CW    # 4 i-chunks
SCALE = 0.12
RMS_EPS = 1.1920929e-07
ROT_BASE = HD * 4


def build_program(repeats: int = 1):
    nc = bacc.Bacc("TRN2", target_bir_lowering=False, debug=False,
                   num_devices=N_CORES)

    xt = nc.dram_tensor("xt", [D, T], BF16, kind="ExternalInput").ap()
    wq = nc.dram_tensor("wq", [D, F], BF16, kind="ExternalInput").ap()
    wk = nc.dram_tensor("wk", [D, F], BF16, kind="ExternalInput").ap()
    wv = nc.dram_tensor("wv", [D, F], BF16, kind="ExternalInput").ap()
    vein = nc.dram_tensor("vein", [T, F], BF16, kind="ExternalInput").ap()
    cpw = nc.dram_tensor("cpw", [F, D], BF16, kind="ExternalInput").ap()
    crep = nc.dram_tensor("crep", [F, T], BF16, kind="ExternalInput").ap()
    srep = nc.dram_tensor("srep", [F, T], BF16, kind="ExternalInput").ap()
    perm = nc.dram_tensor("perm", [128, 128], BF16, kind="ExternalInput").ap()
    selq = nc.dram_tensor("selq", [128, 128], BF16, kind="ExternalInput").ap()
    selk = nc.dram_tensor("selk", [128, 128], BF16, kind="ExternalInput").ap()
    selqb = nc.dram_tensor("selqb", [128, 128], BF16, kind="ExternalInput").ap()
    selkb = nc.dram_tensor("selkb", [128, 128], BF16, kind="ExternalInput").ap()
    trimask = nc.dram_tensor("trimask", [128, 128], BF16,
                             kind="ExternalInput").ap()
    out = nc.dram_tensor("out", [T, D], F32, kind="ExternalOutput").ap()
    dbg = {}
    if os.environ.get("KDBG"):
        for nm, shp in (("d_qT", [128, T]), ("d_kT", [128, T]),
                        ("d_v", [128, TT * F]), ("d_rs", [128, NCH * CW]),
                        ("d_yT", [128, T])):
            dbg[nm] = nc.dram_tensor(nm, shp, F32, kind="ExternalOutput").ap()

    with tile.TileContext(nc) as tc, ExitStack() as ctx:
        const = ctx.enter_context(tc.tile_pool(name="const", bufs=1))
        x_sb = const.tile([128, DT, T], BF16)
        wq_sb = const.tile([128, DT, F], BF16)
        wk_sb = const.tile([128, DT, F], BF16)
        wv_sb = const.tile([128, DT, F], BF16)
        ve_sb = const.tile([128, TT, F], BF16)
        cpw_sb = const.tile([128, D], BF16)
        crep_sb = const.tile([128, T], BF16)
        srep_sb = const.tile([128, T], BF16)
        perm_sb = const.tile([128, 128], BF16)
        selq_sb = const.tile([128, 128], BF16)
        selk_sb = const.tile([128, 128], BF16)
        selqb_sb = const.tile([128, 128], BF16)
        selkb_sb = const.tile([128, 128], BF16)
        tri_sb = const.tile([128, 128], BF16)
        ones_sb = const.tile([128, 1], BF16)
        eps_sb = const.tile([128, 1], F32)
        qT_sb = const.tile([128, T], BF16)
        kT_sb = const.tile([128, T], BF16)
        v_sb = const.tile([128, TT, F], BF16)
        yT_sb = const.tile([128, T], BF16)

        def body(_iv=None):
            # ---- constant loads ----
            xr = xt.rearrange("(n p) t -> p n t", p=128)
            for c in range(NCH):
                nc.sync.dma_start(x_sb[:, :, CW * c:CW * (c + 1)],
                                  xr[:, :, CW * c:CW * (c + 1)])
            nc.sync.dma_start(wq_sb[:], wq.rearrange("(n p) f -> p n f", p=128))
            nc.sync.dma_start(wk_sb[:], wk.rearrange("(n p) f -> p n f", p=128))
            nc.sync.dma_start(wv_sb[:], wv.rearrange("(n p) f -> p n f", p=128))
            nc.sync.dma_start(ve_sb[:], vein.rearrange("(n p) f -> p n f", p=128))
            nc.sync.dma_start(cpw_sb[:], cpw)
            nc.sync.dma_start(crep_sb[:], crep)
            nc.sync.dma_start(srep_sb[:], srep)
            nc.sync.dma_start(perm_sb[:], perm)
            nc.sync.dma_start(selq_sb[:], selq)
            nc.sync.dma_start(selk_sb[:], selk)
            nc.sync.dma_start(selqb_sb[:], selqb)
            nc.sync.dma_start(selkb_sb[:], selkb)
            nc.sync.dma_start(tri_sb[:], trimask)
            nc.vector.memset(ones_sb[:], 1.0)
            nc.vector.memset(eps_sb[:], RMS_EPS)

            # ---- QKV phase: per 512-col chunk ----
            rs_sb = const.tile([128, NCH, CW], BF16)
            lnr_sb = const.tile([128, NCH, CW], F32)
            with tc.tile_pool(name="qk_ps", bufs=1, space="PSUM") as qk_ps, \
                 tc.tile_pool(name="ms_ps", bufs=1, space="PSUM") as ms_ps_p, \
                 tc.tile_pool(name="sw_ps", bufs=1, space="PSUM") as sw_ps, \
                 tc.tile_pool(name="rep_ps", bufs=1, space="PSUM") as rep_ps, \
                 tc.tile_pool(name="qwork", bufs=2) as qwork:
                for c in range(NCH):
                    cs = slice(CW * c, CW * (c + 1))
                    q_ps = qk_ps.tile([128, CW], F32, tag="q")
                    k_ps = qk_ps.tile([128, CW], F32, tag="k")
                    v_ps = qk_ps.tile([128, HC, 128], F32, tag="v")
                    for dt in range(DT):
                        nc.tensor.matmul(q_ps[:], lhsT=wq_sb[:, dt, :],
                                         rhs=x_sb[:, dt, cs],
                                         start=(dt == 0), stop=(dt == DT - 1))
                    for dt in range(DT):
                        nc.tensor.matmul(k_ps[:], lhsT=wk_sb[:, dt, :],
                                         rhs=x_sb[:, dt, cs],
                                         start=(dt == 0), stop=(dt == DT - 1))
                    for st in range(HC):
                        t0 = CW * c + 128 * st
                        for dt in range(DT):
                            nc.tensor.matmul(
                                v_ps[:, st, :],
                                lhsT=x_sb[:, dt, t0:t0 + 128],
                                rhs=wv_sb[:, dt, :],
                                start=(dt == 0), stop=(dt == DT - 1))

                    # v merge: wv pre-scaled by lam0, ve by lam1 on host
                    nc.vector.tensor_add(
                        v_sb[:, HC * c:HC * (c + 1), :],
                        v_ps[:], ve_sb[:, HC * c:HC * (c + 1), :])

                    # raw q,k to SBUF (bf16)
                    qraw = qwork.tile([128, CW], BF16, tag="qraw")
                    kraw = qwork.tile([128, CW], BF16, tag="kraw")
                    nc.vector.tensor_copy(qraw[:], q_ps[:])
                    nc.vector.tensor_copy(kraw[:], k_ps[:])

                    # squared values packed [q | k]
                    sq = qwork.tile([128, 2 * CW], BF16, tag="sq")
                    nc.vector.tensor_mul(sq[:, 0:CW], qraw[:], qraw[:])
                    nc.vector.tensor_mul(sq[:, CW:2 * CW], kraw[:], kraw[:])

                    # partition-reduce within heads: ms[h, :] (q cols 0:CW,
                    # k cols CW:2CW)
                    ms_ps = ms_ps_p.tile([128, CW], F32, tag="ms")
                    nc.tensor.matmul(ms_ps[:], lhsT=selq_sb[:],
                                     rhs=sq[:, 0:CW], start=True, stop=False)
                    nc.tensor.matmul(ms_ps[:], lhsT=selk_sb[:],
                                     rhs=sq[:, CW:2 * CW],
                                     start=False, stop=True)

                    # rstd = exp(-0.5 * ln(ms/HD + eps))
                    # Ln only here; Exp batched after the loop so ACT
                    # needs just two table loads in the whole program.
                    nc.scalar.activation(lnr_sb[:, c, :], ms_ps[:], AF.Ln,
                                         bias=eps_sb[:], scale=1.0 / HD)

                    # rotary: swap-by-16 via PE permutation matmul
                    qsw_ps = sw_ps.tile([128, CW], F32, tag="qsw")
                    ksw_ps = sw_ps.tile([128, CW], F32, tag="ksw")
                    nc.tensor.matmul(qsw_ps[:], lhsT=perm_sb[:], rhs=qraw[:],
                                     start=True, stop=True)
                    nc.tensor.matmul(ksw_ps[:], lhsT=perm_sb[:], rhs=kraw[:],
                                     start=True, stop=True)

                    for raw, swp, dst in ((qraw, qsw_ps, qT_sb),
                                          (kraw, ksw_ps, kT_sb)):
                        ta = qwork.tile([128, CW], BF16, tag="ta")
                        tb = qwork.tile([128, CW], BF16, tag="tb")
                        nc.vector.tensor_mul(ta[:], raw[:], crep_sb[:, cs])
                        nc.vector.tensor_mul(tb[:], swp[:], srep_sb[:, cs])
                        # unnormalized rotated q/k; rstd applied after loop
                        nc.vector.tensor_add(dst[:, cs], ta[:], tb[:])

                # one Exp over all chunks' Ln results (single table set use)
                nc.scalar.activation(
                    rs_sb[:].rearrange("p a b -> p (a b)"),
                    lnr_sb[:].rearrange("p a b -> p (a b)"),
                    AF.Exp, scale=-0.5)

                # apply rstd: replicate via PE selector matmuls, multiply in
                for c in range(NCH):
                    cs = slice(CW * c, CW * (c + 1))
                    rep_q = rep_ps.tile([128, CW], F32, tag="rep_q")
                    rep_k = rep_ps.tile([128, CW], F32, tag="rep_k")
                    nc.tensor.matmul(rep_q[:], lhsT=selqb_sb[:],
                                     rhs=rs_sb[:, c, :], start=True, stop=True)
                    nc.tensor.matmul(rep_k[:], lhsT=selkb_sb[:],
                                     rhs=rs_sb[:, c, :], start=True, stop=True)
                    nc.vector.tensor_mul(qT_sb[:, cs], qT_sb[:, cs], rep_q[:])
                    nc.vector.tensor_mul(kT_sb[:, cs], kT_sb[:, cs], rep_k[:])

            # ---- attention + c_proj, per i-chunk ----
            with tc.tile_pool(name="sc_ps", bufs=2, space="PSUM") as sc_ps, \
                 tc.tile_pool(name="y_ps", bufs=1, space="PSUM") as y_ps_p, \
                 tc.tile_pool(name="r_ps", bufs=1, space="PSUM") as r_ps_p, \
                 tc.tile_pool(name="o_ps", bufs=1, space="PSUM") as o_ps_p, \
                 tc.tile_pool(name="rrb_ps", bufs=1, space="PSUM") as rrb_ps_p, \
                 tc.tile_pool(name="pt", bufs=4) as pt_pool, \
                 tc.tile_pool(name="nrm", bufs=2) as nrm, \
                 tc.tile_pool(name="o_sb", bufs=4) as o_sb:
                for c in range(NCH):
                    y_ps = y_ps_p.tile([128, CW], F32, tag="y")
                    r_ps = r_ps_p.tile([128, CW], F32, tag="r")
                    nc.vector.memset(r_ps[:], 1.0)
                    njt = 4 * c + 4
                    for jt in range(njt):
                        q0 = max(0, jt - 4 * c)
                        nt = CW - 128 * q0
                        i_lo = CW * c + 128 * q0
                        i_hi = CW * (c + 1)
                        diag = jt >= 4 * c
                        for p in range(2):
                            sc = sc_ps.tile([128, 2, CW], F32, tag="sc")
                            pt = pt_pool.tile([128, 2, CW], BF16, tag="pt")
                            for hh in range(2):
                                h = 2 * p + hh
                                nc.tensor.matmul(
                                    sc[:, hh, 0:nt],
                                    lhsT=kT_sb[32 * h:32 * (h + 1),
                                               128 * jt:128 * (jt + 1)],
                                    rhs=qT_sb[32 * h:32 * (h + 1), i_lo:i_hi],
                                    start=True, stop=True,
                                    tile_position=(32 * h, 0))
                            nc.scalar.activation(pt[:, :, 0:nt],
                                                 sc[:, :, 0:nt],
                                                 AF.Exp, scale=SCALE)
                            if diag:
                                for hh in range(2):
                                    nc.gpsimd.tensor_mul(
                                        pt[:, hh, 0:128],
                                        pt[:, hh, 0:128],
                                        tri_sb[:])
                            for hh in range(2):
                                h = 2 * p + hh
                                nc.tensor.matmul(
                                    y_ps[32 * h:32 * (h + 1), 128 * q0:CW],
                                    lhsT=v_sb[:, jt, 32 * h:32 * (h + 1)],
                                    rhs=pt[:, hh, 0:nt],
                                    start=(jt == 0), stop=(jt == njt - 1),
                                    tile_position=(0, 32 * h),
                                    skip_group_check=True)
                                nc.tensor.matmul(
                                    r_ps[32 * h:32 * h + 1, 128 * q0:CW],
                                    lhsT=ones_sb[:],
                                    rhs=pt[:, hh, 0:nt],
                                    start=(jt == 0), stop=(jt == njt - 1),
                                    tile_position=(0, 32 * h),
                                    skip_group_check=True)

                    # normalize: yT = y * (1/r) broadcast within heads
                    rr = nrm.tile([128, CW], BF16, tag="rr")
                    with nc.allow_low_precision(reason="softmax denom bf16"):
                        nc.vector.reciprocal(rr[:], r_ps[:])
                    rrb_ps = rrb_ps_p.tile([128, CW], F32, tag="rrb")
                    nc.tensor.matmul(rrb_ps[:], lhsT=selqb_sb[:], rhs=rr[:],
                                     start=True, stop=True)
                    rrb = nrm.tile([128, CW], BF16, tag="rrb")
                    nc.vector.tensor_copy(rrb[:], rrb_ps[:])
                    nc.vector.tensor_mul(yT_sb[:, CW * c:CW * (c + 1)],
                                         y_ps[:], rrb[:])

                    # c_proj for this i-chunk
                    for ti in range(4):
                        tg = 4 * c + ti
                        for dc in range(2):
                            op = o_ps_p.tile([128, CW], F32, tag="op")
                            nc.tensor.matmul(
                                op[:],
                                lhsT=yT_sb[:, 128 * tg:128 * (tg + 1)],
                                rhs=cpw_sb[:, CW * dc:CW * (dc + 1)],
                                start=True, stop=True)
                            ob = o_sb.tile([128, CW], F32, tag="ob")
                            nc.vector.tensor_copy(ob[:], op[:])
                            nc.sync.dma_start(
                                out[128 * tg:128 * (tg + 1),
                                    CW * dc:CW * (dc + 1)], ob[:])

            if dbg:
                with tc.tile_pool(name="dbgp", bufs=1) as dbgp:
                    for nm, sb in (("d_qT", qT_sb), ("d_kT", kT_sb),
                                   ("d_yT", yT_sb)):
                        dt_ = dbgp.tile([128, T], F32, tag=nm)
                        nc.vector.tensor_copy(dt_[:], sb[:])
                        nc.sync.dma_start(dbg[nm], dt_[:])
                    dt_ = dbgp.tile([128, TT, F], F32, tag="dv")
                    nc.vector.tensor_copy(dt_[:], v_sb[:])
                    nc.sync.dma_start(
                        dbg["d_v"].rearrange("p (a b) -> p a b", a=TT), dt_[:])
                    dt_ = dbgp.tile([128, NCH, CW], F32, tag="drs")
                    nc.vector.tensor_copy(dt_[:], rs_sb[:])
                    nc.sync.dma_start(
                        dbg["d_rs"].rearrange("p (a b) -> p a b", a=NCH),
                        dt_[:])

        if repeats == 1:
            body()
        else:
            with tc.For_i(0, repeats, 1) as _i:
                body(_i)

    nc.compile()
    return nc


def _host_inputs(x, ve, qkv_w, lambdas, c_proj_w):
    bf = ml_dtypes.bfloat16
    x2 = np.asarray(x, np.float32).reshape(T, D)
    xt = np.ascontiguousarray(x2.T).astype(bf)
    ve2 = np.asarray(ve, np.float32).reshape(T, D)
    qkv = np.asarray(qkv_w, np.float32)
    cp = np.asarray(c_proj_w, np.float32)
    lamv = np.asarray(lambdas, np.float32).reshape(2)

    # rotary tables in [f, t] layout, replicated per head
    quarter = HD // 4
    af = (np.float32(1.0) / np.float32(ROT_BASE)) ** np.linspace(
        0.0, 1.0, quarter, dtype=np.float32)
    af = np.concatenate([af, np.zeros(quarter, np.float32)])  # [16]
    theta = np.arange(T, dtype=np.float32)[:, None] * af[None, :]  # [T, 16]
    cos = np.cos(theta).astype(np.float32)   # [T, 16]
    sin = np.sin(theta).astype(np.float32)
    # crep[p, t] = cos(theta[t, p % 16]); srep sign-flipped on second half
    ph = np.arange(128) % HD                 # position within head
    crep = cos[:, ph % 16].T                 # [128, T]
    ssign = np.where(ph < 16, 1.0, -1.0).astype(np.float32)
    srep = (sin[:, ph % 16] * ssign[None, :]).T
    crep = np.ascontiguousarray(crep).astype(bf)
    srep = np.ascontiguousarray(srep).astype(bf)

    # swap-by-16 permutation within each 32-row head block:
    # out[p] = in[(p//32)*32 + ((p%32)+16)%32]  -> perm[j, p] = 1 iff j maps
    pm = np.zeros((128, 128), np.float32)
    for p in range(128):
        j = (p // 32) * 32 + ((p % 32) + 16) % 32
        pm[j, p] = 1.0
    perm = pm.astype(bf)

    # selectors: ms-reduce q->row 32h, k->row 32h+16; b = broadcast back
    sq_ = np.zeros((128, 128), np.float32)
    sk_ = np.zeros((128, 128), np.float32)
    for p in range(128):
        sq_[p, 32 * (p // 32)] = 1.0
        sk_[p, 32 * (p // 32) + 16] = 1.0
    selq_, selk_ = sq_.astype(bf), sk_.astype(bf)
    selqb_ = np.ascontiguousarray(sq_.T).astype(bf)
    selkb_ = np.ascontiguousarray(sk_.T).astype(bf)

    jj = np.arange(128)[:, None]
    ii = np.arange(128)[None, :]
    trim = (ii >= jj).astype(bf)                       # keep j <= i

    in_maps = []
    for cidx in range(N_CORES):
        sl = slice(F * cidx, F * (cidx + 1))
        wqh = np.ascontiguousarray(qkv[0][sl, :].T).astype(bf)  # [D, F]
        wkh = np.ascontiguousarray(qkv[1][sl, :].T).astype(bf)
        wvh = np.ascontiguousarray(
            (lamv[0] * qkv[2][sl, :]).T).astype(bf)
        in_maps.append({
            "xt": xt,
            "wq": wqh,
            "wk": wkh,
            "wv": wvh,
            "vein": np.ascontiguousarray(lamv[1] * ve2[:, sl]).astype(bf),
            "cpw": np.ascontiguousarray(cp[:, sl].T).astype(bf),
            "crep": crep,
            "srep": srep,
            "perm": perm,
            "selq": selq_,
            "selk": selk_,
            "selqb": selqb_,
            "selkb": selkb_,
            "trimask": trim,
        })
    return in_maps


_NC_CACHE = {}


def _get_program(repeats: int = 1):
    if repeats not in _NC_CACHE:
        _NC_CACHE[repeats] = build_program(repeats)
    return _NC_CACHE[repeats]


def run_prepared(in_maps, repeats: int = 1):
    nc = _get_program(repeats)
    res = run_bass_kernel_spmd(nc, in_maps, list(range(N_CORES)))
    acc = np.zeros((T, D), np.float64)
    for cidx in range(N_CORES):
        acc += res.results[cidx]["out"].astype(np.float64)
    return acc.astype(np.float32).reshape(1, T, D)


def run(inputs, repeats: int = 1):
    in_maps = _host_inputs(**inputs)
    return run_prepared(in_maps, repeats)


def kernel(x, ve, qkv_w, lambdas, c_proj_w):
    return run(dict(x=x, ve=ve, qkv_w=qkv_w, lambdas=lambdas,
                    c_proj_w=c_proj_w))
